# revision 12
# baseline (speedup 1.0000x reference)
"""BiLSTM tagger on 8 Trainium2 NeuronCores.

Strategy (hardcoded for B=64, S=1024, E=128, H=256, TAGS=50, VOCAB=50000):
  - Data-parallel over batch: 8 sequences per core; each core runs its fwd and
    bwd LSTM chains interleaved so one chain's gate math hides under the other
    chain's PE matmuls.
  - Channel-partition layout: gates are computed transposed (gate-dim on SBUF
    partitions, batch on the free dim) so ACT/DVE use all 128 lanes.
  - The input GEMM (Wx @ e) is fused into the recurrence as a third K-tile of
    the per-step matmul, with embeddings held transposed in SBUF (gathered by
    indirect DMA + PE-transposed once). Wx matmuls are batched 4 steps per
    PSUM accumulation group to amortize weight loads.
  - Gate rows are host-reordered to [i, f, o, g] so one sigmoid covers i,f,o.
  - Final FC runs on-device from a bf16 transposed h-staging buffer; each core
    emits logits for its own 8 sequences; the host just concatenates.
"""

import contextlib
import ctypes
import os
import sys
import types

import numpy as np

for _p in ("/opt/trn_rl_repo",):
    if os.path.isdir(_p) and _p not in sys.path:
        sys.path.insert(0, _p)

B, S_FULL, E, H, VOCAB, TAGS = 64, 1024, 128, 256, 50000, 50
NCORES = 8
BL = B // NCORES            # sequences per core (per direction chain)
NM = (4 * H) // 128         # 8 gate-row tiles of 128
QUAD = 8                    # steps per Wx PSUM accumulation group
S = int(os.environ.get("KERNEL_DEV_S", str(S_FULL)))  # dev-only override
NG = (BL * S) // 128        # embedding gathers per chain
GSTEPS = 128 // BL          # steps covered per gather (16)

MM_NP = np.float32          # matmul-path host dtype (np.float32 | ml_dtypes.bfloat16)

LAST_EXEC_TIME_NS = None
_LAST_RES = None
_NC = None


def _mm_np():
    if os.environ.get("KERNEL_BF16", "0") == "1":
        import ml_dtypes

        return ml_dtypes.bfloat16
    return np.float32


def _install_ntff_hook():
    """Make bass_utils' axon NTFF profiling work: provide antenv.axon_hooks."""
    if "antenv.axon_hooks" in sys.modules:
        return
    so_path = "/opt/axon/libaxon_pjrt.so"
    try:
        lib = ctypes.CDLL(so_path)
        lib.axon_start_nrt_profile.argtypes = [
            ctypes.POINTER(ctypes.c_int64),
            ctypes.c_size_t,
        ]
        lib.axon_start_nrt_profile.restype = ctypes.c_int64
        lib.axon_stop_nrt_profile.argtypes = [ctypes.c_char_p]
        lib.axon_stop_nrt_profile.restype = ctypes.c_int64
    except (OSError, AttributeError):
        return

    @contextlib.contextmanager
    def _hook(output_dir, device_ids):
        import jax

        jax.devices()
        if device_ids:
            ids = (ctypes.c_int64 * len(device_ids))(*device_ids)
            rc = lib.axon_start_nrt_profile(ids, len(device_ids))
        else:
            rc = lib.axon_start_nrt_profile(None, 0)
        if rc != 0:
            raise RuntimeError(f"axon_start_nrt_profile rc={rc}")
        try:
            yield
        finally:
            n = lib.axon_stop_nrt_profile(str(output_dir).encode())
            if n <= 0:
                print(f"ntff profile capture wrote {n} files to {output_dir}")

    mod = types.ModuleType("antenv.axon_hooks")
    mod.get_axon_ntff_profile_hook = lambda: _hook
    mod.set_axon_ntff_profile_hook = lambda h: None
    sys.modules["antenv.axon_hooks"] = mod


def _build():
    import concourse.bass as bass
    import concourse.mybir as mybir
    from concourse import bacc
    from concourse.tile import TileContext

    f32 = mybir.dt.float32
    bf16 = mybir.dt.bfloat16
    i32 = mybir.dt.int32
    mmdt = bf16 if _mm_np() != np.float32 else f32
    sig = mybir.ActivationFunctionType.Sigmoid
    tanh = mybir.ActivationFunctionType.Tanh
    mul = mybir.AluOpType.mult

    nc = bacc.Bacc("TRN2", target_bir_lowering=False, debug=False, num_devices=NCORES)

    emb_d = nc.dram_tensor("emb", [VOCAB, E], mmdt, kind="ExternalInput").ap()
    idx_d = [
        nc.dram_tensor(f"idx{d}", [128, NG], i32, kind="ExternalInput").ap()
        for d in range(2)
    ]
    w_d = [
        nc.dram_tensor(f"w{d}", [128, 3 * NM * 128], mmdt, kind="ExternalInput").ap()
        for d in range(2)
    ]
    bias_d = [
        nc.dram_tensor(f"bias{d}", [1, NM * 128], f32, kind="ExternalInput").ap()
        for d in range(2)
    ]
    wfc_d = nc.dram_tensor("wfc", [128, 4 * TAGS], bf16, kind="ExternalInput").ap()
    bfc_d = nc.dram_tensor("bfc", [128, TAGS], f32, kind="ExternalInput").ap()
    ident_d = nc.dram_tensor("ident", [128, 128], mmdt, kind="ExternalInput").ap()
    out_d = nc.dram_tensor("logits", [BL * S, TAGS], f32, kind="ExternalOutput").ap()
    dbg = os.environ.get("KERNEL_DEBUG_DUMP", "0") == "1"
    if dbg:
        et_out = [
            nc.dram_tensor(f"et_out{d}", [128, BL * S], mmdt, kind="ExternalOutput").ap()
            for d in range(2)
        ]
        stg_out = [
            nc.dram_tensor(f"stg_out{d}", [128, 2 * BL * S], bf16, kind="ExternalOutput").ap()
            for d in range(2)
        ]
        act_out = [
            nc.dram_tensor(f"act_out{d}", [128, NM * BL], f32, kind="ExternalOutput").ap()
            for d in range(2)
        ]

    with TileContext(nc) as tc, contextlib.ExitStack() as es:
        const = es.enter_context(tc.tile_pool(name="const", bufs=1))
        W = [const.tile([128, 3 * NM * 128], mmdt, tag=f"W{d}", name=f"W{d}") for d in range(2)]
        BIAS = [const.tile([1, NM * 128], f32, tag=f"bias{d}", name=f"bias{d}") for d in range(2)]
        ONES = const.tile([1, QUAD * BL], f32, tag="ones", name="ones")
        IDX = [const.tile([128, NG], i32, tag=f"idx{d}", name=f"idx{d}") for d in range(2)]
        WFC = const.tile([128, 4 * TAGS], bf16, tag="wfc", name="wfc")
        BFC = const.tile([128, TAGS], f32, tag="bfc", name="bfc")
        IDENT = const.tile([128, 128], mmdt, tag="ident", name="ident")
        ET = [const.tile([128, BL * S], mmdt, tag=f"eT{d}", name=f"eT{d}") for d in range(2)]
        STG = [const.tile([128, 2 * BL * S], bf16, tag=f"stg{d}", name=f"stg{d}") for d in range(2)]
        C = [const.tile([128, 2 * BL], f32, tag=f"c{d}", name=f"c{d}") for d in range(2)]
        H0 = [const.tile([128, 2 * BL], mmdt, tag=f"h0{d}", name=f"h0{d}") for d in range(2)]

        for d in range(2):
            nc.sync.dma_start(out=W[d][:], in_=w_d[d])
            nc.sync.dma_start(out=BIAS[d][:], in_=bias_d[d])
            nc.sync.dma_start(out=IDX[d][:], in_=idx_d[d])
        nc.sync.dma_start(out=WFC[:], in_=wfc_d)
        nc.sync.dma_start(out=BFC[:], in_=bfc_d)
        nc.sync.dma_start(out=IDENT[:], in_=ident_d)
        for d in range(2):
            nc.gpsimd.memset(C[d][:], 0.0)
            nc.gpsimd.memset(H0[d][:], 0.0)
        nc.gpsimd.memset(ONES[:], 1.0)

        gat = es.enter_context(tc.tile_pool(name="gat", bufs=4))
        use_dmat = (mmdt == bf16) and os.environ.get("KERNEL_DMAT", "1") == "1"
        if not use_dmat:
            tps = es.enter_context(tc.tile_pool(name="tps", bufs=2, space="PSUM"))
        psp = [
            es.enter_context(tc.tile_pool(name=f"ps{d}", bufs=2, space="PSUM"))
            for d in range(2)
        ]
        wk = es.enter_context(tc.tile_pool(name="wk", bufs=2))

        def emit_gather(g, d):
            eg = gat.tile([128, E], mmdt, tag=f"eg{d}", name=f"eg{d}")
            nc.gpsimd.indirect_dma_start(
                out=eg[:],
                out_offset=None,
                in_=emb_d,
                in_offset=bass.IndirectOffsetOnAxis(ap=IDX[d][:, g : g + 1], axis=0),
            )
            dst = ET[d][:, g * 128 : (g + 1) * 128]
            if use_dmat:
                nc.sync.dma_start_transpose(out=dst, in_=eg[:])
            else:
                tp = tps.tile([128, 128], mmdt, tag="tp", name="tp")
                nc.tensor.transpose(out=tp[:], in_=eg[:], identity=IDENT[:])
                nc.vector.tensor_copy(out=dst, in_=tp[:])

        QB = QUAD * BL
        stg_r = [
            STG[d][:].rearrange("p (half j s) -> p half j s", half=2, s=S)
            for d in range(2)
        ]

        def quad_prep_ops(d, q):
            """Matmul thunks filling quad q's psum with bias + Wx."""
            ops = []
            for m in range(NM):
                def op(m=m, d=d, q=q):
                    nc.tensor.matmul(
                        out=psum_for[d][q % 2][:, m * QB : (m + 1) * QB],
                        lhsT=BIAS[d][:, m * 128 : (m + 1) * 128],
                        rhs=ONES[:],
                        start=(m == 0),
                        stop=False,
                        skip_group_check=True,
                    )
                ops.append(op)
            for m in range(NM):
                def op(m=m, d=d, q=q):
                    nc.tensor.matmul(
                        out=psum_for[d][q % 2][:, m * QB : (m + 1) * QB],
                        lhsT=W[d][:, 2 * NM * 128 + m * 128 : 2 * NM * 128 + (m + 1) * 128],
                        rhs=ET[d][:, q * QB : (q + 1) * QB],
                        start=False,
                        stop=False,
                        skip_group_check=True,
                    )
                ops.append(op)
            return ops

        # two psum slots per chain, reallocated per quad parity
        psum_for = [[None, None], [None, None]]
        prep_queue = [[], []]

        def alloc_quad(d, q):
            psum_for[d][q % 2] = psp[d].tile(
                [128, NM * QB], f32, tag=f"gps{d}", name=f"gps{d}"
            )

        def emit_mm(d, t):
            q, r = divmod(t, QUAD)
            ps = psum_for[d][q % 2]
            if t == 0:
                rhs_h = [H0[d][:, 0:BL], H0[d][:, BL : 2 * BL]]
            else:
                pprev = t - 1 if d == 0 else S - t
                rhs_h = [stg_r[d][:, 0, :, pprev], stg_r[d][:, 1, :, pprev]]
            # g-gate tiles (m6, m7) first so tanh(g) overlaps the i/f/o matmuls
            for m in (6, 7, 0, 1, 2, 3, 4, 5):
                reg = ps[:, m * QB + r * BL : m * QB + (r + 1) * BL]
                nc.tensor.matmul(
                    out=reg, lhsT=W[d][:, m * 128 : (m + 1) * 128], rhs=rhs_h[0],
                    start=False, stop=False, skip_group_check=True,
                )
                nc.tensor.matmul(
                    out=reg,
                    lhsT=W[d][:, NM * 128 + m * 128 : NM * 128 + (m + 1) * 128],
                    rhs=rhs_h[1],
                    start=False, stop=True, skip_group_check=True,
                )
            # dribble out next quad's bias/Wx matmuls (2 per step, off critical path)
            for _ in range(2):
                if prep_queue[d]:
                    prep_queue[d].pop(0)()

        heads = [None, None]

        def emit_math_head(d, t):
            q, r = divmod(t, QUAD)
            ps_r = psum_for[d][q % 2][:].rearrange(
                "p (m r j) -> p m r j", r=QUAD, j=BL
            )
            act = wk.tile([128, NM * BL], f32, tag=f"act{d}", name=f"act{d}")
            nc.scalar.activation(
                out=act[:, 6 * BL : 8 * BL], in_=ps_r[:, 6:8, r, :], func=tanh
            )
            nc.scalar.activation(
                out=act[:, 0 : 6 * BL], in_=ps_r[:, 0:6, r, :], func=sig
            )
            tmp = wk.tile([128, 2 * BL], f32, tag=f"tmp{d}", name=f"tmp{d}")
            nc.vector.tensor_tensor(
                out=C[d][:], in0=act[:, 2 * BL : 4 * BL], in1=C[d][:], op=mul
            )
            nc.vector.tensor_tensor(
                out=tmp[:], in0=act[:, 0 : 2 * BL], in1=act[:, 6 * BL : 8 * BL], op=mul
            )
            nc.vector.tensor_add(out=C[d][:], in0=C[d][:], in1=tmp[:])
            heads[d] = act

        def emit_math_tail(d, t):
            act = heads[d]
            th = wk.tile([128, 2 * BL], f32, tag=f"th{d}", name=f"th{d}")
            nc.scalar.activation(out=th[:], in_=C[d][:], func=tanh)
            pos = t if d == 0 else S - 1 - t
            st = stg_r[d][:, :, :, pos]
            nc.vector.tensor_tensor(
                out=st, in0=act[:, 4 * BL : 6 * BL], in1=th[:], op=mul
            )
            if dbg and t == 0:
                nc.sync.dma_start(out=act_out[d], in_=act[:])

        NQ = S // QUAD
        emit_gather(0, 0)
        emit_gather(0, 1)
        for d in range(2):
            alloc_quad(d, 0)
            for op in quad_prep_ops(d, 0):
                op()

        for t in range(S):
            q, r = divmod(t, QUAD)
            if t % GSTEPS == GSTEPS // 2:
                g = (t + GSTEPS // 2) // GSTEPS
                if g < NG:
                    emit_gather(g, 0)
                    emit_gather(g, 1)
            if r == 0 and q + 1 < NQ:
                for d in range(2):
                    alloc_quad(d, q + 1)
                    prep_queue[d].extend(quad_prep_ops(d, q + 1))
            emit_mm(0, t)
            if t > 0:
                emit_math_tail(1, t - 1)
            emit_math_head(0, t)
            emit_mm(1, t)
            emit_math_tail(0, t)
            emit_math_head(1, t)
        emit_math_tail(1, S - 1)

        # FC: logits[128 rows of (j, pos), 50] per M-tile
        fco = es.enter_context(tc.tile_pool(name="fco", bufs=3))
        n_mt = (BL * S) // 128
        for jt in range(n_mt):
            ps = psp[0].tile([128, TAGS], f32, tag="gps0", name="fps")
            for k in range(4):
                d, half = divmod(k, 2)
                lhsT = STG[d][:, half * BL * S + jt * 128 : half * BL * S + (jt + 1) * 128]
                nc.tensor.matmul(
                    out=ps[:],
                    lhsT=lhsT,
                    rhs=WFC[:, k * TAGS : (k + 1) * TAGS],
                    start=(k == 0),
                    stop=(k == 3),
                )
            fo = fco.tile([128, TAGS], f32, tag="fo", name="fo")
            nc.vector.tensor_add(out=fo[:], in0=ps[:], in1=BFC[:])
            nc.sync.dma_start(out=out_d[jt * 128 : (jt + 1) * 128, :], in_=fo[:])

    nc.compile()
    return nc


def _prep_shared(emb, Wx_f, bx_f, Wh_f, bh_f, Wx_b, bx_b, Wh_b, bh_b, Wfc, bfc):
    mmnp = _mm_np()
    import ml_dtypes

    perm = np.concatenate(
        [np.arange(0, 512), np.arange(768, 1024), np.arange(512, 768)]
    )

    def wpack(Wx, Wh):
        WaT = np.concatenate([Wh[perm].T, Wx[perm].T], axis=0)  # [384, 4H]
        return np.ascontiguousarray(
            WaT.reshape(3, 128, NM, 128).transpose(1, 0, 2, 3).reshape(128, 3 * NM * 128)
        ).astype(mmnp)

    def bpack(bx, bh):
        return np.ascontiguousarray((bx + bh)[perm].reshape(1, NM * 128)).astype(
            np.float32
        )

    shared = {
        "emb": np.ascontiguousarray(emb).astype(mmnp),
        "w0": wpack(Wx_f, Wh_f),
        "w1": wpack(Wx_b, Wh_b),
        "bias0": bpack(bx_f, bh_f),
        "bias1": bpack(bx_b, bh_b),
        "wfc": np.ascontiguousarray(
            Wfc.T.reshape(4, 128, TAGS).transpose(1, 0, 2).reshape(128, 4 * TAGS)
        ).astype(ml_dtypes.bfloat16),
        "bfc": np.tile(bfc.astype(np.float32), (128, 1)),
        "ident": np.eye(128, dtype=np.float32).astype(mmnp),
    }
    return shared


def kernel(x, lengths, emb, Wx_f, bx_f, Wh_f, bh_f, Wx_b, bx_b, Wh_b, bh_b, Wfc, bfc):
    global LAST_EXEC_TIME_NS, _LAST_RES, _NC
    from concourse import bass_utils

    x = np.asarray(x)[:, :S].astype(np.int32)
    shared = _prep_shared(
        np.asarray(emb, np.float32),
        np.asarray(Wx_f, np.float32),
        np.asarray(bx_f, np.float32),
        np.asarray(Wh_f, np.float32),
        np.asarray(bh_f, np.float32),
        np.asarray(Wx_b, np.float32),
        np.asarray(bx_b, np.float32),
        np.asarray(Wh_b, np.float32),
        np.asarray(bh_b, np.float32),
        np.asarray(Wfc, np.float32),
        np.asarray(bfc, np.float32),
    )

    in_maps = []
    for core in range(NCORES):
        xi = x[core * BL : (core + 1) * BL]  # [BL, S]
        idx0 = np.ascontiguousarray(xi.T.reshape(NG, 128).T).astype(np.int32)
        idx1 = np.ascontiguousarray(
            xi[:, ::-1].T.reshape(NG, 128).T
        ).astype(np.int32)
        in_maps.append({**shared, "idx0": idx0, "idx1": idx1})

    profile = os.environ.get("KERNEL_PROFILE", "0") == "1"
    if profile:
        _install_ntff_hook()

    if _NC is None:
        _NC = _build()
    res = bass_utils.run_bass_kernel_spmd(
        _NC, in_maps, list(range(NCORES)), trace=profile
    )
    LAST_EXEC_TIME_NS = res.exec_time_ns
    _LAST_RES = res

    parts = [
        res.results[i]["logits"].reshape(BL, S, TAGS) for i in range(NCORES)
    ]
    return np.concatenate(parts, axis=0).astype(np.float32)


# revision 15
# speedup vs baseline: 9.1531x; 9.1531x over previous
"""BiLSTM tagger on 8 Trainium2 NeuronCores.

Strategy (hardcoded for B=64, S=1024, E=128, H=256, TAGS=50, VOCAB=50000):
  - Data-parallel over batch: 8 sequences per core; each core runs its fwd and
    bwd LSTM chains interleaved so one chain's gate math hides under the other
    chain's PE matmuls.
  - Channel-partition layout: gates are computed transposed (gate-dim on SBUF
    partitions, batch on the free dim) so ACT/DVE use all 128 lanes.
  - The input GEMM (Wx @ e) is fused into the recurrence as a third K-tile of
    the per-step matmul, with embeddings held transposed in SBUF (gathered by
    indirect DMA + PE-transposed once). Wx matmuls are batched 4 steps per
    PSUM accumulation group to amortize weight loads.
  - Gate rows are host-reordered to [i, f, o, g] so one sigmoid covers i,f,o.
  - Final FC runs on-device from a bf16 transposed h-staging buffer; each core
    emits logits for its own 8 sequences; the host just concatenates.
"""

import contextlib
import ctypes
import os
import sys
import types

import numpy as np

for _p in ("/opt/trn_rl_repo",):
    if os.path.isdir(_p) and _p not in sys.path:
        sys.path.insert(0, _p)

B, S_FULL, E, H, VOCAB, TAGS = 64, 1024, 128, 256, 50000, 50
NCORES = 8
BL = B // NCORES            # sequences per core (per direction chain)
NM = (4 * H) // 128         # 8 gate-row tiles of 128
QUAD = 8                    # steps per Wx PSUM accumulation group
S = int(os.environ.get("KERNEL_DEV_S", str(S_FULL)))  # dev-only override
NG = (BL * S) // 128        # embedding gathers per chain
GSTEPS = 128 // BL          # steps covered per gather (16)

MM_NP = np.float32          # matmul-path host dtype (np.float32 | ml_dtypes.bfloat16)

LAST_EXEC_TIME_NS = None
_LAST_RES = None
_NC = None


def _mm_np():
    if os.environ.get("KERNEL_BF16", "0") == "1":
        import ml_dtypes

        return ml_dtypes.bfloat16
    return np.float32


def _install_ntff_hook():
    """Make bass_utils' axon NTFF profiling work: provide antenv.axon_hooks."""
    if "antenv.axon_hooks" in sys.modules:
        return
    so_path = "/opt/axon/libaxon_pjrt.so"
    try:
        lib = ctypes.CDLL(so_path)
        lib.axon_start_nrt_profile.argtypes = [
            ctypes.POINTER(ctypes.c_int64),
            ctypes.c_size_t,
        ]
        lib.axon_start_nrt_profile.restype = ctypes.c_int64
        lib.axon_stop_nrt_profile.argtypes = [ctypes.c_char_p]
        lib.axon_stop_nrt_profile.restype = ctypes.c_int64
    except (OSError, AttributeError):
        return

    @contextlib.contextmanager
    def _hook(output_dir, device_ids):
        import jax

        jax.devices()
        if device_ids:
            ids = (ctypes.c_int64 * len(device_ids))(*device_ids)
            rc = lib.axon_start_nrt_profile(ids, len(device_ids))
        else:
            rc = lib.axon_start_nrt_profile(None, 0)
        if rc != 0:
            raise RuntimeError(f"axon_start_nrt_profile rc={rc}")
        try:
            yield
        finally:
            n = lib.axon_stop_nrt_profile(str(output_dir).encode())
            if n <= 0:
                print(f"ntff profile capture wrote {n} files to {output_dir}")

    mod = types.ModuleType("antenv.axon_hooks")
    mod.get_axon_ntff_profile_hook = lambda: _hook
    mod.set_axon_ntff_profile_hook = lambda h: None
    sys.modules["antenv.axon_hooks"] = mod


def _build():
    import concourse.bass as bass
    import concourse.mybir as mybir
    from concourse import bacc
    from concourse.tile import TileContext

    f32 = mybir.dt.float32
    bf16 = mybir.dt.bfloat16
    i32 = mybir.dt.int32
    mmdt = bf16 if _mm_np() != np.float32 else f32
    sig = mybir.ActivationFunctionType.Sigmoid
    tanh = mybir.ActivationFunctionType.Tanh
    mul = mybir.AluOpType.mult

    nc = bacc.Bacc("TRN2", target_bir_lowering=False, debug=False, num_devices=NCORES)

    emb_d = nc.dram_tensor("emb", [VOCAB, E], mmdt, kind="ExternalInput").ap()
    idx_d = [
        nc.dram_tensor(f"idx{d}", [128, NG], i32, kind="ExternalInput").ap()
        for d in range(2)
    ]
    w_d = [
        nc.dram_tensor(f"w{d}", [128, 3 * NM * 128], mmdt, kind="ExternalInput").ap()
        for d in range(2)
    ]
    bias_d = [
        nc.dram_tensor(f"bias{d}", [1, 2 * NM * 128], bf16, kind="ExternalInput").ap()
        for d in range(2)
    ]
    wfc_d = nc.dram_tensor("wfc", [128, 4 * TAGS], bf16, kind="ExternalInput").ap()
    bfc_d = nc.dram_tensor("bfc", [128, TAGS], f32, kind="ExternalInput").ap()
    ident_d = nc.dram_tensor("ident", [128, 128], mmdt, kind="ExternalInput").ap()
    out_d = nc.dram_tensor("logits", [BL * S, TAGS], f32, kind="ExternalOutput").ap()
    dbg = os.environ.get("KERNEL_DEBUG_DUMP", "0") == "1"
    if dbg:
        et_out = [
            nc.dram_tensor(f"et_out{d}", [128, BL * S], mmdt, kind="ExternalOutput").ap()
            for d in range(2)
        ]
        stg_out = [
            nc.dram_tensor(f"stg_out{d}", [128, 2 * BL * S], bf16, kind="ExternalOutput").ap()
            for d in range(2)
        ]
        act_out = [
            nc.dram_tensor(f"act_out{d}", [128, NM * BL], f32, kind="ExternalOutput").ap()
            for d in range(2)
        ]

    with TileContext(nc) as tc, contextlib.ExitStack() as es:
        const = es.enter_context(tc.tile_pool(name="const", bufs=1))
        W = [const.tile([128, 3 * NM * 128], mmdt, tag=f"W{d}", name=f"W{d}") for d in range(2)]
        BIAS = [const.tile([1, 2 * NM * 128], bf16, tag=f"bias{d}", name=f"bias{d}") for d in range(2)]
        ONES = const.tile([1, QUAD * BL], bf16, tag="ones", name="ones")
        IDX = [const.tile([128, NG], i32, tag=f"idx{d}", name=f"idx{d}") for d in range(2)]
        WFC = const.tile([128, 4 * TAGS], bf16, tag="wfc", name="wfc")
        BFC = const.tile([128, TAGS], f32, tag="bfc", name="bfc")
        IDENT = const.tile([128, 128], mmdt, tag="ident", name="ident")
        ET = [const.tile([128, BL * S], mmdt, tag=f"eT{d}", name=f"eT{d}") for d in range(2)]
        STG = [const.tile([128, 2 * BL * S], bf16, tag=f"stg{d}", name=f"stg{d}") for d in range(2)]
        C = [const.tile([128, 2 * BL], f32, tag=f"c{d}", name=f"c{d}") for d in range(2)]
        H0 = [const.tile([128, 2 * BL], mmdt, tag=f"h0{d}", name=f"h0{d}") for d in range(2)]

        for d in range(2):
            nc.sync.dma_start(out=W[d][:], in_=w_d[d])
            nc.sync.dma_start(out=BIAS[d][:], in_=bias_d[d])
            nc.sync.dma_start(out=IDX[d][:], in_=idx_d[d])
        nc.sync.dma_start(out=WFC[:], in_=wfc_d)
        nc.sync.dma_start(out=BFC[:], in_=bfc_d)
        nc.sync.dma_start(out=IDENT[:], in_=ident_d)
        for d in range(2):
            nc.gpsimd.memset(C[d][:], 0.0)
            nc.gpsimd.memset(H0[d][:], 0.0)
        nc.gpsimd.memset(ONES[:], 1.0)

        gat = es.enter_context(tc.tile_pool(name="gat", bufs=4))
        use_dmat = (mmdt == bf16) and os.environ.get("KERNEL_DMAT", "1") == "1"
        if not use_dmat:
            tps = es.enter_context(tc.tile_pool(name="tps", bufs=2, space="PSUM"))
        psp = [
            es.enter_context(tc.tile_pool(name=f"ps{d}", bufs=2, space="PSUM"))
            for d in range(2)
        ]
        wk = es.enter_context(tc.tile_pool(name="wk", bufs=2))

        def emit_gather(g, d):
            eg = gat.tile([128, E], mmdt, tag=f"eg{d}", name=f"eg{d}")
            nc.gpsimd.indirect_dma_start(
                out=eg[:],
                out_offset=None,
                in_=emb_d,
                in_offset=bass.IndirectOffsetOnAxis(ap=IDX[d][:, g : g + 1], axis=0),
            )
            dst = ET[d][:, g * 128 : (g + 1) * 128]
            if use_dmat:
                nc.sync.dma_start_transpose(out=dst, in_=eg[:])
            else:
                tp = tps.tile([128, 128], mmdt, tag="tp", name="tp")
                nc.tensor.transpose(out=tp[:], in_=eg[:], identity=IDENT[:])
                nc.vector.tensor_copy(out=dst, in_=tp[:])

        QB = QUAD * BL
        stg_r = [
            STG[d][:].rearrange("p (s half j) -> p s half j", half=2, j=BL)
            for d in range(2)
        ]

        def quad_prep_ops(d, q):
            """Matmul thunks filling quad q's psum with bias + Wx."""
            ops = []
            for i in range(2 * NM):
                def op(i=i, d=d, q=q):
                    nc.tensor.matmul(
                        out=psum_for[d][q % 2][
                            :, (i % NM) * QB : (i % NM + 1) * QB
                        ],
                        lhsT=BIAS[d][:, i * 128 : (i + 1) * 128],
                        rhs=ONES[:],
                        start=(i == 0),
                        stop=False,
                        skip_group_check=True,
                    )
                ops.append(op)
            for m in range(NM):
                def op(m=m, d=d, q=q):
                    nc.tensor.matmul(
                        out=psum_for[d][q % 2][:, m * QB : (m + 1) * QB],
                        lhsT=W[d][:, 2 * NM * 128 + m * 128 : 2 * NM * 128 + (m + 1) * 128],
                        rhs=ET[d][:, q * QB : (q + 1) * QB],
                        start=False,
                        stop=False,
                        skip_group_check=True,
                    )
                ops.append(op)
            return ops

        # two psum slots per chain, reallocated per quad parity
        psum_for = [[None, None], [None, None]]
        prep_queue = [[], []]

        def alloc_quad(d, q):
            psum_for[d][q % 2] = psp[d].tile(
                [128, NM * QB], f32, tag=f"gps{d}", name=f"gps{d}"
            )

        def emit_mm(d, t):
            q, r = divmod(t, QUAD)
            ps = psum_for[d][q % 2]
            if t == 0:
                rhs_h = [H0[d][:, 0:BL], H0[d][:, BL : 2 * BL]]
            else:
                pprev = t - 1 if d == 0 else S - t
                rhs_h = [stg_r[d][:, pprev, 0, :], stg_r[d][:, pprev, 1, :]]
            # g-gate tiles (m6, m7) first so tanh(g) overlaps the i/f/o matmuls
            for m in (6, 7, 0, 1, 2, 3, 4, 5):
                reg = ps[:, m * QB + r * BL : m * QB + (r + 1) * BL]
                nc.tensor.matmul(
                    out=reg, lhsT=W[d][:, m * 128 : (m + 1) * 128], rhs=rhs_h[0],
                    start=False, stop=False, skip_group_check=True,
                )
                nc.tensor.matmul(
                    out=reg,
                    lhsT=W[d][:, NM * 128 + m * 128 : NM * 128 + (m + 1) * 128],
                    rhs=rhs_h[1],
                    start=False, stop=True, skip_group_check=True,
                )
            # dribble out next quad's bias/Wx matmuls (3 per step, off critical path)
            for _ in range(3):
                if prep_queue[d]:
                    prep_queue[d].pop(0)()

        heads = [None, None]

        def emit_math_head(d, t):
            q, r = divmod(t, QUAD)
            ps_r = psum_for[d][q % 2][:].rearrange(
                "p (m r j) -> p m r j", r=QUAD, j=BL
            )
            act = wk.tile([128, NM * BL], f32, tag=f"act{d}", name=f"act{d}")
            nc.scalar.activation(
                out=act[:, 6 * BL : 8 * BL], in_=ps_r[:, 6:8, r, :], func=tanh
            )
            nc.scalar.activation(
                out=act[:, 0 : 6 * BL], in_=ps_r[:, 0:6, r, :], func=sig
            )
            tmp = wk.tile([128, 2 * BL], f32, tag=f"tmp{d}", name=f"tmp{d}")
            nc.vector.tensor_tensor(
                out=C[d][:], in0=act[:, 2 * BL : 4 * BL], in1=C[d][:], op=mul
            )
            nc.vector.tensor_tensor(
                out=tmp[:], in0=act[:, 0 : 2 * BL], in1=act[:, 6 * BL : 8 * BL], op=mul
            )
            nc.vector.tensor_add(out=C[d][:], in0=C[d][:], in1=tmp[:])
            heads[d] = act

        def emit_math_tail(d, t):
            act = heads[d]
            th = wk.tile([128, 2 * BL], f32, tag=f"th{d}", name=f"th{d}")
            nc.scalar.activation(out=th[:], in_=C[d][:], func=tanh)
            pos = t if d == 0 else S - 1 - t
            st = stg_r[d][:, pos, :, :]
            nc.vector.tensor_tensor(
                out=st, in0=act[:, 4 * BL : 6 * BL], in1=th[:], op=mul
            )
            if dbg and t == 0:
                nc.sync.dma_start(out=act_out[d], in_=act[:])

        NQ = S // QUAD
        emit_gather(0, 0)
        emit_gather(0, 1)
        for d in range(2):
            alloc_quad(d, 0)
            for op in quad_prep_ops(d, 0):
                op()

        for t in range(S):
            q, r = divmod(t, QUAD)
            if t % GSTEPS == GSTEPS // 2:
                g = (t + GSTEPS // 2) // GSTEPS
                if g < NG:
                    emit_gather(g, 0)
                    emit_gather(g, 1)
            if r == 0 and q + 1 < NQ:
                for d in range(2):
                    alloc_quad(d, q + 1)
                    prep_queue[d].extend(quad_prep_ops(d, q + 1))
            emit_mm(0, t)
            if t > 0:
                emit_math_tail(1, t - 1)
            emit_math_head(0, t)
            emit_mm(1, t)
            emit_math_tail(0, t)
            emit_math_head(1, t)
        emit_math_tail(1, S - 1)

        # FC: logits[128 rows of (j, pos), 50] per M-tile
        fco = es.enter_context(tc.tile_pool(name="fco", bufs=3))
        n_mt = (BL * S) // 128
        for jt in range(n_mt):
            ps = psp[0].tile([128, TAGS], f32, tag="gps0", name="fps")
            for k in range(4):
                d, half = divmod(k, 2)
                sbl = S // 128
                lhsT = STG[d][:].rearrange(
                    "p (s half j) -> p j s half", half=2, j=BL
                )[:, jt // sbl, (jt % sbl) * 128 : (jt % sbl + 1) * 128, half]
                nc.tensor.matmul(
                    out=ps[:],
                    lhsT=lhsT,
                    rhs=WFC[:, k * TAGS : (k + 1) * TAGS],
                    start=(k == 0),
                    stop=(k == 3),
                )
            fo = fco.tile([128, TAGS], f32, tag="fo", name="fo")
            nc.vector.tensor_add(out=fo[:], in0=ps[:], in1=BFC[:])
            nc.sync.dma_start(out=out_d[jt * 128 : (jt + 1) * 128, :], in_=fo[:])

    nc.compile()
    return nc


def _prep_shared(emb, Wx_f, bx_f, Wh_f, bh_f, Wx_b, bx_b, Wh_b, bh_b, Wfc, bfc):
    import ml_dtypes

    mmnp = _mm_np()

    perm = np.concatenate(
        [np.arange(0, 512), np.arange(768, 1024), np.arange(512, 768)]
    )

    def wpack(Wx, Wh):
        WaT = np.concatenate([Wh[perm].T, Wx[perm].T], axis=0)  # [384, 4H]
        return np.ascontiguousarray(
            WaT.reshape(3, 128, NM, 128).transpose(1, 0, 2, 3).reshape(128, 3 * NM * 128)
        ).astype(mmnp)

    def bpack(bx, bh):
        b = (bx + bh)[perm].astype(np.float32)
        hi = b.astype(ml_dtypes.bfloat16)
        lo = (b - hi.astype(np.float32)).astype(ml_dtypes.bfloat16)
        return np.ascontiguousarray(
            np.concatenate([hi, lo]).reshape(1, 2 * NM * 128)
        )

    shared = {
        "emb": np.ascontiguousarray(emb).astype(mmnp),
        "w0": wpack(Wx_f, Wh_f),
        "w1": wpack(Wx_b, Wh_b),
        "bias0": bpack(bx_f, bh_f),
        "bias1": bpack(bx_b, bh_b),
        "wfc": np.ascontiguousarray(
            Wfc.T.reshape(4, 128, TAGS).transpose(1, 0, 2).reshape(128, 4 * TAGS)
        ).astype(ml_dtypes.bfloat16),
        "bfc": np.tile(bfc.astype(np.float32), (128, 1)),
        "ident": np.eye(128, dtype=np.float32).astype(mmnp),
    }
    return shared


def kernel(x, lengths, emb, Wx_f, bx_f, Wh_f, bh_f, Wx_b, bx_b, Wh_b, bh_b, Wfc, bfc):
    global LAST_EXEC_TIME_NS, _LAST_RES, _NC
    from concourse import bass_utils

    x = np.asarray(x)[:, :S].astype(np.int32)
    shared = _prep_shared(
        np.asarray(emb, np.float32),
        np.asarray(Wx_f, np.float32),
        np.asarray(bx_f, np.float32),
        np.asarray(Wh_f, np.float32),
        np.asarray(bh_f, np.float32),
        np.asarray(Wx_b, np.float32),
        np.asarray(bx_b, np.float32),
        np.asarray(Wh_b, np.float32),
        np.asarray(bh_b, np.float32),
        np.asarray(Wfc, np.float32),
        np.asarray(bfc, np.float32),
    )

    in_maps = []
    for core in range(NCORES):
        xi = x[core * BL : (core + 1) * BL]  # [BL, S]
        idx0 = np.ascontiguousarray(xi.T.reshape(NG, 128).T).astype(np.int32)
        idx1 = np.ascontiguousarray(
            xi[:, ::-1].T.reshape(NG, 128).T
        ).astype(np.int32)
        in_maps.append({**shared, "idx0": idx0, "idx1": idx1})

    profile = os.environ.get("KERNEL_PROFILE", "0") == "1"
    if profile:
        _install_ntff_hook()

    if _NC is None:
        _NC = _build()
    res = bass_utils.run_bass_kernel_spmd(
        _NC, in_maps, list(range(NCORES)), trace=profile
    )
    LAST_EXEC_TIME_NS = res.exec_time_ns
    _LAST_RES = res

    parts = [
        res.results[i]["logits"].reshape(BL, S, TAGS) for i in range(NCORES)
    ]
    return np.concatenate(parts, axis=0).astype(np.float32)
